# revision 41
# baseline (speedup 1.0000x reference)
"""Trainium2 Bass kernel for MinibatchDiscrimination — v5 (shift-packed pairs).

Math:
    M = (x @ T.reshape(512, 320)).reshape(1024, 64, 5)
    dist[i, j, f] = sum_k |M[i, f, k] - M[j, f, k]|
    out[i, f] = sum_j exp(-dist[i, j, f])            # (1024, 64)

Strategy (8 cores, SPMD): dist is symmetric, so each core computes, for
each of its 128 rows i (global u = 128c + r), only the SLIDING
half-window of pairs j in [u, u+512).  The relu identity
|d| = 2 relu(d) - d turns the k-sum into matmuls; the -SM_j/2 term
rides a static sliding tile (smp2) and -SM_i enters as the ACT exp
bias.  Raw exp tiles stream to HBM; the host does the banded
transpose-sum and adds the gap-512 diagonal pairs (u, u+512).

v5 packs ROW PAIRS into the partition axis: tile AS_k holds k-plane
data on partitions (rho, f) with the rho=1 half shifted one column, so
one tensor_scalar [128, 512] computes relu windows for rows r (rho=0)
and r+1 (rho=1) at once — 5 DVE ops per PAIR instead of 3 per row
(DVE was the loop bottleneck: ~130 ns fixed cost per instruction).
PSUM holds both rows as [128 = (rho, f), 512 = j'], accumulated by six
identity matmuls (5 k-planes + smp2), and ONE exp per pair covers both
rows with a per-partition bias (negsm2 column p').  The loop drops from
~720 ns/row (3 ops/row DVE floor) to ~648 ns/row — the new DVE floor:
320 maximal [128, 512] tensor_scalar ops is provably minimal for the
N^2/2 x 320-value pair workload (every instruction is full-width with
zero partition waste).  Inputs ship in head (pair-0-critical columns)
plus deferred tail DMAs; outputs batch 4 pairs on the sync queue with a
finer-grained final flush to shorten the drain tail.  The device window
is WD=480 of the 512 deltas: the self term (exactly 1.0) and the thin
edge diagonals delta in [481, 512] are sewn in on the host alongside
the existing banded transpose-sum, trimming every loop engine ~6%.
"""

import numpy as np
import ml_dtypes

import concourse.bass as bass
import concourse.bacc as bacc
import concourse.mybir as mybir
import concourse.tile as tile
from concourse import bass_utils

BF16 = ml_dtypes.bfloat16

N, IN_F, OUT_F, KD = 1024, 512, 64, 5
NCORES = 8
ROWS = N // NCORES          # 128 rows per core
R = OUT_F * KD              # 320 MT rows, r = k*64 + f
W = 512                     # full pair window per row (device + host bands)
WD = 480                    # device window: deltas [1, 481); host does the rest
LC2 = ROWS + W + 2          # 642 local columns held per core (shift spare)
PAIRS = ROWS // 2           # 64 row pairs per core

_COMPILED = None


def _build_program():
    nc = bacc.Bacc("TRN2", target_bir_lowering=False, debug=False,
                   num_devices=NCORES)
    dt = mybir.dt
    alu = mybir.AluOpType
    AF = mybir.ActivationFunctionType

    as_d = [nc.dram_tensor(f"as{k}", [128, LC2], dt.bfloat16,
                           kind="ExternalInput").ap() for k in range(KD)]
    sm_d = nc.dram_tensor("smp2", [128, LC2], dt.bfloat16,
                          kind="ExternalInput").ap()
    idn_d = nc.dram_tensor("idn", [128, 128], dt.bfloat16,
                           kind="ExternalInput").ap()
    mk_d = nc.dram_tensor("mtsk", [128, 384], dt.bfloat16,
                          kind="ExternalInput").ap()
    ng_d = nc.dram_tensor("negsm2", [128, 64], dt.float32,
                          kind="ExternalInput").ap()
    esc_d = nc.dram_tensor("escout", [128, PAIRS * WD], dt.bfloat16,
                           kind="ExternalOutput").ap()

    with tile.TileContext(nc) as tc:
        with (
            tc.tile_pool(name="persist", bufs=1) as pp,
            tc.tile_pool(name="work", bufs=6) as rp,
            tc.tile_pool(name="escp", bufs=2) as ep,
            tc.tile_pool(name="psB", bufs=4, space="PSUM") as psB,
        ):
            # inputs on both queues, pair-0-critical head columns first;
            # the tail columns (needed from pair ~5 on) stream afterwards
            HD = 516
            # small pair-0-critical tensors first on each queue, then the
            # big head tiles; smp2 feeds the LAST matmul of the group so it
            # may land latest without gating exp 0
            ng = pp.tile([128, 64], dt.float32, tag="ng", name="ng")
            nc.sync.dma_start(ng[:], ng_d[:])
            mkb = pp.tile([128, 384], dt.bfloat16, tag="mkb", name="mkb")
            nc.scalar.dma_start(mkb[:], mk_d[:])
            idn_sb = pp.tile([128, 128], dt.bfloat16, tag="idn", name="idn")
            nc.scalar.dma_start(idn_sb[:], idn_d[:])
            AS = []
            for k in range(KD):
                t = pp.tile([128, LC2], dt.bfloat16, tag=f"as{k}",
                            name=f"as{k}")
                eng = nc.sync if k % 2 == 0 else nc.scalar
                eng.dma_start(t[:, 0:HD], as_d[k][:, 0:HD])
                AS.append(t)
            smp2 = pp.tile([128, LC2], dt.bfloat16, tag="smp2", name="smp2")
            nc.sync.dma_start(smp2[:, 0:HD], sm_d[:, 0:HD])
            mkf = pp.tile([128, 384], dt.float32, tag="mkf", name="mkf")
            nc.scalar.activation(mkf[:], mkb[:], AF.Copy, bias=0.0, scale=1.0)

            mtsK = [mkf[:, 64 * k:64 * k + 64] for k in range(KD)]
            negsm2 = ng[:]

            esc = None
            for p in range(PAIRS):
                r = 2 * p
                bb = rp.tile([128, KD * WD], dt.bfloat16, tag="bb", name="bb")
                for k in range(KD - 1):
                    nc.vector.tensor_scalar(
                        out=bb[:, k * WD:(k + 1) * WD],
                        in0=AS[k][:, r + 1:r + 1 + WD],
                        scalar1=mtsK[k][:, p:p + 1], scalar2=0.0,
                        op0=alu.subtract, op1=alu.max)
                # k4 relu runs on the half-idle ACT engine: relu(in - m)
                # via the per-partition bias slot (negated scalars block)
                nc.scalar.activation(
                    bb[:, 4 * WD:5 * WD], AS[4][:, r + 1:r + 1 + WD],
                    AF.Relu, bias=mkf[:, 320 + p:321 + p], scale=1.0)

                ps2 = psB.tile([128, WD], dt.float32, tag="psB", name="psB")
                for k in range(KD):
                    nc.tensor.matmul(ps2[:], lhsT=idn_sb[:],
                                     rhs=bb[:, k * WD:(k + 1) * WD],
                                     start=(k == 0), stop=False,
                                     skip_group_check=True)
                nc.tensor.matmul(ps2[:], lhsT=idn_sb[:],
                                 rhs=smp2[:, r + 1:r + 1 + WD],
                                 start=False, stop=True, skip_group_check=True)

                if p == 2:
                    # tail columns: issued once the head pairs are in flight
                    for k in range(KD):
                        eng = nc.sync if k % 2 == 0 else nc.scalar
                        eng.dma_start(AS[k][:, 516:LC2], as_d[k][:, 516:LC2])
                    nc.sync.dma_start(smp2[:, 516:LC2], sm_d[:, 516:LC2])
                if p % 4 == 0:
                    esc = ep.tile([128, 4 * WD], dt.bfloat16, tag="esc",
                                  name="esc")
                nc.scalar.activation(
                    esc[:, (p % 4) * WD:(p % 4 + 1) * WD], ps2[:], AF.Exp,
                    bias=negsm2[:, p:p + 1], scale=-2.0)
                if p == PAIRS - 3:
                    nc.sync.dma_start(
                        esc_d[:, 60 * WD:62 * WD], esc[:, 0:2 * WD])
                if p == PAIRS - 2:
                    nc.sync.dma_start(
                        esc_d[:, 62 * WD:63 * WD], esc[:, 2 * WD:3 * WD])
                if p == PAIRS - 1:
                    nc.sync.dma_start(
                        esc_d[:, 63 * WD:64 * WD], esc[:, 3 * WD:4 * WD])
                elif p % 4 == 3:
                    t4 = p // 4
                    nc.sync.dma_start(esc_d[:, t4 * 4 * WD:(t4 + 1) * 4 * WD],
                                      esc[:])

    nc.compile()
    return nc


def _host_inputs(x, T):
    """Full-input host prep: MT = (x @ T2)^T is tiny (336 MFLOPs) next to
    the O(N^2) pair work, so it and all derived static tiles are computed
    here, letting the device start its main loop straight after the DMAs."""
    t2r = T.transpose(0, 2, 1).reshape(IN_F, R).astype(np.float32)
    MT = np.ascontiguousarray((x.astype(np.float32) @ t2r).T)    # (320, 1024)
    MTb = MT.astype(BF16)

    idn = np.eye(128, dtype=np.float32).astype(BF16)
    # SM = sum_k MT_k from the bf16 values; smhalf = bf16(-SM/2);
    # negsm2 = 2*smhalf exactly so the self term cancels to exp(0).
    SMg = MTb.astype(np.float32).reshape(KD, OUT_F, N).sum(axis=0)
    smh = (-0.5 * SMg).astype(BF16)                  # (64, 1024) bf16

    in_maps = []
    for c in range(NCORES):
        cols = (c * ROWS + np.arange(LC2)) % N
        cols1 = (cols + 1) % N
        m = {}
        for k in range(KD):
            blk = MTb[k * OUT_F:(k + 1) * OUT_F]     # (64, 1024)
            ask = np.empty((128, LC2), dtype=BF16)
            ask[0:64] = blk[:, cols]                 # rho = 0
            ask[64:128] = blk[:, cols1]              # rho = 1 (shift by one)
            m[f"as{k}"] = ask
        smp2 = np.empty((128, LC2), dtype=BF16)
        smp2[0:64] = smh[:, cols]
        smp2[64:128] = smh[:, cols1]
        m["smp2"] = smp2
        m["idn"] = idn
        # fp32 per-partition scalars: mtsK columns p -> M[2p+rho, f, k];
        # negsm2 column p -> 2*smhalf at the pair's self columns.
        pc = (c * ROWS + 2 * np.arange(PAIRS)) % N   # rho=0 self cols
        pc1 = (pc + 1) % N                           # rho=1 self cols
        mkb = np.empty((128, 384), dtype=BF16)
        for k in range(KD):
            blk = MTb[k * OUT_F:(k + 1) * OUT_F]
            mkb[0:64, 64 * k:64 * k + 64] = blk[:, pc]
            mkb[64:128, 64 * k:64 * k + 64] = blk[:, pc1]
        mkb[:, 320:384] = -mkb[:, 256:320]            # negated k4 block
        m["mtsk"] = mkb
        sh32 = smh.astype(np.float32)
        ng = np.empty((128, 64), dtype=np.float32)
        ng[0:64] = 2.0 * sh32[:, pc]
        ng[64:128] = 2.0 * sh32[:, pc1]
        m["negsm2"] = ng
        in_maps.append(m)
    return in_maps, MTb


def _assemble(results, MTb):
    out = np.zeros((N, OUT_F), dtype=np.float32)
    for c in range(NCORES):
        E = results[c]["escout"].astype(np.float32)  # (128, PAIRS*WD)
        E = E.reshape(2, OUT_F, PAIRS, WD)           # (rho, f, p, j')
        # direct side: row sums over the window; row r = 2p + rho,
        # window deltas [1, WD+1) (no self term on device)
        rows = E.sum(axis=3)                         # (rho, f, p)
        out[c * ROWS + 0:c * ROWS + ROWS:2] += rows[0].T
        out[c * ROWS + 1:c * ROWS + ROWS:2] += rows[1].T
        # transpose side: banded column sums at local col l = r + 1 + j'
        contrib = np.zeros((LC2, OUT_F), dtype=np.float32)
        for p in range(PAIRS):
            contrib[2 * p + 1:2 * p + 1 + WD] += E[0, :, p, :].T
            contrib[2 * p + 2:2 * p + 2 + WD] += E[1, :, p, :].T
        jidx = (c * ROWS + np.arange(LC2)) % N
        np.add.at(out, jidx, contrib)
    # host-side diagonal bands: delta = 0 (self term, exactly 1.0),
    # deltas [WD+1, 512) two-sided, and the self-symmetric delta = 512
    out += 1.0
    Mf = MTb.astype(np.float32)                      # (320, 1024)
    for d in range(WD + 1, W):
        D = np.abs(Mf - np.roll(Mf, -d, axis=1))
        eb = np.exp(-D.reshape(KD, OUT_F, N).sum(axis=0))    # (64, N) at row u
        out += eb.T
        out += np.roll(eb, d, axis=1).T              # partner row u + d
    D = np.abs(Mf - np.roll(Mf, -W, axis=1))
    out += np.exp(-D.reshape(KD, OUT_F, N).sum(axis=0)).T
    return np.ascontiguousarray(out, dtype=np.float32)


def _ensure_ntff_hook():
    """The agent image's antenv lacks axon_hooks; shim it so trace=True
    works (bass_utils imports antenv.axon_hooks unconditionally)."""
    import sys
    import types
    try:
        from antenv import axon_hooks  # noqa: F401
        return
    except ImportError:
        pass
    mod = types.ModuleType("antenv.axon_hooks")
    holder = [None]
    mod.set_axon_ntff_profile_hook = lambda h: holder.__setitem__(0, h)
    mod.get_axon_ntff_profile_hook = lambda: holder[0]
    import antenv
    antenv.axon_hooks = mod
    sys.modules["antenv.axon_hooks"] = mod
    try:
        from trn_agent_boot.trn_boot import _ntff_profile_via_ctypes
        h = _ntff_profile_via_ctypes("/opt/axon/libaxon_pjrt.so")
        if h is not None:
            mod.set_axon_ntff_profile_hook(h)
    except Exception:
        pass


def _get_compiled():
    global _COMPILED
    if _COMPILED is None:
        _COMPILED = _build_program()
    return _COMPILED


def kernel(x, T, _trace=False):
    if _trace:
        _ensure_ntff_hook()
    nc = _get_compiled()
    in_maps, MTb = _host_inputs(np.asarray(x, dtype=np.float32),
                                np.asarray(T, dtype=np.float32))
    res = bass_utils.run_bass_kernel_spmd(nc, in_maps,
                                          core_ids=list(range(NCORES)),
                                          trace=_trace)
    out = _assemble(res.results, MTb)
    if _trace:
        return out, res
    return out


# revision 42
# speedup vs baseline: 1.1639x; 1.1639x over previous
"""Trainium2 Bass kernel for MinibatchDiscrimination — v5 (shift-packed pairs).

Math:
    M = (x @ T.reshape(512, 320)).reshape(1024, 64, 5)
    dist[i, j, f] = sum_k |M[i, f, k] - M[j, f, k]|
    out[i, f] = sum_j exp(-dist[i, j, f])            # (1024, 64)

Strategy (8 cores, SPMD): dist is symmetric, so each core computes, for
each of its 128 rows i (global u = 128c + r), only the SLIDING
half-window of pairs j in [u, u+512).  The relu identity
|d| = 2 relu(d) - d turns the k-sum into matmuls; the -SM_j/2 term
rides a static sliding tile (smp2) and -SM_i enters as the ACT exp
bias.  Raw exp tiles stream to HBM; the host does the banded
transpose-sum and adds the gap-512 diagonal pairs (u, u+512).

v5 packs ROW PAIRS into the partition axis: tile AS_k holds k-plane
data on partitions (rho, f) with the rho=1 half shifted one column, so
one tensor_scalar [128, 512] computes relu windows for rows r (rho=0)
and r+1 (rho=1) at once — 5 DVE ops per PAIR instead of 3 per row
(DVE was the loop bottleneck: ~130 ns fixed cost per instruction).
PSUM holds both rows as [128 = (rho, f), 512 = j'], accumulated by six
identity matmuls (5 k-planes + smp2), and ONE exp per pair covers both
rows with a per-partition bias (negsm2 column p').  The loop drops from
~720 ns/row (3 ops/row DVE floor) to ~648 ns/row — the new DVE floor:
320 maximal [128, 512] tensor_scalar ops is provably minimal for the
N^2/2 x 320-value pair workload (every instruction is full-width with
zero partition waste).  Inputs ship in head (pair-0-critical columns)
plus deferred tail DMAs; outputs batch 4 pairs on the sync queue with a
finer-grained final flush to shorten the drain tail.  The device window
is WD=480 of the 512 deltas: the self term (exactly 1.0) and the thin
edge diagonals delta in [481, 512] are sewn in on the host alongside
the existing banded transpose-sum, trimming every loop engine ~6%.
"""

import numpy as np
import ml_dtypes

import concourse.bass as bass
import concourse.bacc as bacc
import concourse.mybir as mybir
import concourse.tile as tile
from concourse import bass_utils

BF16 = ml_dtypes.bfloat16

N, IN_F, OUT_F, KD = 1024, 512, 64, 5
NCORES = 8
ROWS = N // NCORES          # 128 rows per core
R = OUT_F * KD              # 320 MT rows, r = k*64 + f
W = 512                     # full pair window per row (device + host bands)
WD = 480                    # device window: deltas [1, 481); host does the rest
LC2 = ROWS + W + 2          # 642 local columns held per core (shift spare)
PAIRS = ROWS // 2           # 64 row pairs per core

_COMPILED = None


def _build_program():
    nc = bacc.Bacc("TRN2", target_bir_lowering=False, debug=False,
                   num_devices=NCORES)
    dt = mybir.dt
    alu = mybir.AluOpType
    AF = mybir.ActivationFunctionType

    as_d = [nc.dram_tensor(f"as{k}", [128, LC2], dt.bfloat16,
                           kind="ExternalInput").ap() for k in range(KD)]
    sm_d = nc.dram_tensor("smp2", [128, LC2], dt.bfloat16,
                          kind="ExternalInput").ap()
    idn_d = nc.dram_tensor("idn", [128, 128], dt.bfloat16,
                           kind="ExternalInput").ap()
    mk_d = nc.dram_tensor("mtsk", [128, 320], dt.bfloat16,
                          kind="ExternalInput").ap()
    ng_d = nc.dram_tensor("negsm2", [128, 64], dt.float32,
                          kind="ExternalInput").ap()
    esc_d = nc.dram_tensor("escout", [128, PAIRS * WD], dt.bfloat16,
                           kind="ExternalOutput").ap()

    with tile.TileContext(nc) as tc:
        with (
            tc.tile_pool(name="persist", bufs=1) as pp,
            tc.tile_pool(name="work", bufs=6) as rp,
            tc.tile_pool(name="escp", bufs=2) as ep,
            tc.tile_pool(name="psB", bufs=4, space="PSUM") as psB,
        ):
            # inputs on both queues, pair-0-critical head columns first;
            # the tail columns (needed from pair ~5 on) stream afterwards
            HD = 516
            # small pair-0-critical tensors first on each queue, then the
            # big head tiles; smp2 feeds the LAST matmul of the group so it
            # may land latest without gating exp 0
            ng = pp.tile([128, 64], dt.float32, tag="ng", name="ng")
            nc.sync.dma_start(ng[:], ng_d[:])
            mkb = pp.tile([128, 320], dt.bfloat16, tag="mkb", name="mkb")
            nc.scalar.dma_start(mkb[:], mk_d[:])
            idn_sb = pp.tile([128, 128], dt.bfloat16, tag="idn", name="idn")
            nc.scalar.dma_start(idn_sb[:], idn_d[:])
            AS = []
            for k in range(KD):
                t = pp.tile([128, LC2], dt.bfloat16, tag=f"as{k}",
                            name=f"as{k}")
                eng = nc.sync if k % 2 == 0 else nc.scalar
                eng.dma_start(t[:, 0:HD], as_d[k][:, 0:HD])
                AS.append(t)
            smp2 = pp.tile([128, LC2], dt.bfloat16, tag="smp2", name="smp2")
            nc.sync.dma_start(smp2[:, 0:HD], sm_d[:, 0:HD])
            mkf = pp.tile([128, 320], dt.float32, tag="mkf", name="mkf")
            nc.scalar.activation(mkf[:], mkb[:], AF.Copy, bias=0.0, scale=1.0)

            mtsK = [mkf[:, 64 * k:64 * k + 64] for k in range(KD)]
            negsm2 = ng[:]

            esc = None
            for p in range(PAIRS):
                r = 2 * p
                bb = rp.tile([128, KD * WD], dt.bfloat16, tag="bb", name="bb")
                for k in range(KD):
                    nc.vector.tensor_scalar(
                        out=bb[:, k * WD:(k + 1) * WD],
                        in0=AS[k][:, r + 1:r + 1 + WD],
                        scalar1=mtsK[k][:, p:p + 1], scalar2=0.0,
                        op0=alu.subtract, op1=alu.max)

                ps2 = psB.tile([128, WD], dt.float32, tag="psB", name="psB")
                for k in range(KD):
                    nc.tensor.matmul(ps2[:], lhsT=idn_sb[:],
                                     rhs=bb[:, k * WD:(k + 1) * WD],
                                     start=(k == 0), stop=False,
                                     skip_group_check=True)
                nc.tensor.matmul(ps2[:], lhsT=idn_sb[:],
                                 rhs=smp2[:, r + 1:r + 1 + WD],
                                 start=False, stop=True, skip_group_check=True)

                if p == 2:
                    # tail columns: issued once the head pairs are in flight
                    for k in range(KD):
                        eng = nc.sync if k % 2 == 0 else nc.scalar
                        eng.dma_start(AS[k][:, 516:LC2], as_d[k][:, 516:LC2])
                    nc.sync.dma_start(smp2[:, 516:LC2], sm_d[:, 516:LC2])
                if p % 4 == 0:
                    esc = ep.tile([128, 4 * WD], dt.bfloat16, tag="esc",
                                  name="esc")
                nc.scalar.activation(
                    esc[:, (p % 4) * WD:(p % 4 + 1) * WD], ps2[:], AF.Exp,
                    bias=negsm2[:, p:p + 1], scale=-2.0)
                if p == PAIRS - 3:
                    nc.sync.dma_start(
                        esc_d[:, 60 * WD:62 * WD], esc[:, 0:2 * WD])
                if p == PAIRS - 2:
                    nc.sync.dma_start(
                        esc_d[:, 62 * WD:63 * WD], esc[:, 2 * WD:3 * WD])
                if p == PAIRS - 1:
                    nc.sync.dma_start(
                        esc_d[:, 63 * WD:64 * WD], esc[:, 3 * WD:4 * WD])
                elif p % 4 == 3:
                    t4 = p // 4
                    nc.sync.dma_start(esc_d[:, t4 * 4 * WD:(t4 + 1) * 4 * WD],
                                      esc[:])

    nc.compile()
    return nc


def _host_inputs(x, T):
    """Full-input host prep: MT = (x @ T2)^T is tiny (336 MFLOPs) next to
    the O(N^2) pair work, so it and all derived static tiles are computed
    here, letting the device start its main loop straight after the DMAs."""
    t2r = T.transpose(0, 2, 1).reshape(IN_F, R).astype(np.float32)
    MT = np.ascontiguousarray((x.astype(np.float32) @ t2r).T)    # (320, 1024)
    MTb = MT.astype(BF16)

    idn = np.eye(128, dtype=np.float32).astype(BF16)
    # SM = sum_k MT_k from the bf16 values; smhalf = bf16(-SM/2);
    # negsm2 = 2*smhalf exactly so the self term cancels to exp(0).
    SMg = MTb.astype(np.float32).reshape(KD, OUT_F, N).sum(axis=0)
    smh = (-0.5 * SMg).astype(BF16)                  # (64, 1024) bf16

    in_maps = []
    for c in range(NCORES):
        cols = (c * ROWS + np.arange(LC2)) % N
        cols1 = (cols + 1) % N
        m = {}
        for k in range(KD):
            blk = MTb[k * OUT_F:(k + 1) * OUT_F]     # (64, 1024)
            ask = np.empty((128, LC2), dtype=BF16)
            ask[0:64] = blk[:, cols]                 # rho = 0
            ask[64:128] = blk[:, cols1]              # rho = 1 (shift by one)
            m[f"as{k}"] = ask
        smp2 = np.empty((128, LC2), dtype=BF16)
        smp2[0:64] = smh[:, cols]
        smp2[64:128] = smh[:, cols1]
        m["smp2"] = smp2
        m["idn"] = idn
        # fp32 per-partition scalars: mtsK columns p -> M[2p+rho, f, k];
        # negsm2 column p -> 2*smhalf at the pair's self columns.
        pc = (c * ROWS + 2 * np.arange(PAIRS)) % N   # rho=0 self cols
        pc1 = (pc + 1) % N                           # rho=1 self cols
        mkb = np.empty((128, 320), dtype=BF16)
        for k in range(KD):
            blk = MTb[k * OUT_F:(k + 1) * OUT_F]
            mkb[0:64, 64 * k:64 * k + 64] = blk[:, pc]
            mkb[64:128, 64 * k:64 * k + 64] = blk[:, pc1]
        m["mtsk"] = mkb
        sh32 = smh.astype(np.float32)
        ng = np.empty((128, 64), dtype=np.float32)
        ng[0:64] = 2.0 * sh32[:, pc]
        ng[64:128] = 2.0 * sh32[:, pc1]
        m["negsm2"] = ng
        in_maps.append(m)
    return in_maps, MTb


def _assemble(results, MTb):
    out = np.zeros((N, OUT_F), dtype=np.float32)
    for c in range(NCORES):
        E = results[c]["escout"].astype(np.float32)  # (128, PAIRS*WD)
        E = E.reshape(2, OUT_F, PAIRS, WD)           # (rho, f, p, j')
        # direct side: row sums over the window; row r = 2p + rho,
        # window deltas [1, WD+1) (no self term on device)
        rows = E.sum(axis=3)                         # (rho, f, p)
        out[c * ROWS + 0:c * ROWS + ROWS:2] += rows[0].T
        out[c * ROWS + 1:c * ROWS + ROWS:2] += rows[1].T
        # transpose side: banded column sums at local col l = r + 1 + j'
        contrib = np.zeros((LC2, OUT_F), dtype=np.float32)
        for p in range(PAIRS):
            contrib[2 * p + 1:2 * p + 1 + WD] += E[0, :, p, :].T
            contrib[2 * p + 2:2 * p + 2 + WD] += E[1, :, p, :].T
        jidx = (c * ROWS + np.arange(LC2)) % N
        np.add.at(out, jidx, contrib)
    # host-side diagonal bands: delta = 0 (self term, exactly 1.0),
    # deltas [WD+1, 512) two-sided, and the self-symmetric delta = 512
    out += 1.0
    Mf = MTb.astype(np.float32)                      # (320, 1024)
    for d in range(WD + 1, W):
        D = np.abs(Mf - np.roll(Mf, -d, axis=1))
        eb = np.exp(-D.reshape(KD, OUT_F, N).sum(axis=0))    # (64, N) at row u
        out += eb.T
        out += np.roll(eb, d, axis=1).T              # partner row u + d
    D = np.abs(Mf - np.roll(Mf, -W, axis=1))
    out += np.exp(-D.reshape(KD, OUT_F, N).sum(axis=0)).T
    return np.ascontiguousarray(out, dtype=np.float32)


def _ensure_ntff_hook():
    """The agent image's antenv lacks axon_hooks; shim it so trace=True
    works (bass_utils imports antenv.axon_hooks unconditionally)."""
    import sys
    import types
    try:
        from antenv import axon_hooks  # noqa: F401
        return
    except ImportError:
        pass
    mod = types.ModuleType("antenv.axon_hooks")
    holder = [None]
    mod.set_axon_ntff_profile_hook = lambda h: holder.__setitem__(0, h)
    mod.get_axon_ntff_profile_hook = lambda: holder[0]
    import antenv
    antenv.axon_hooks = mod
    sys.modules["antenv.axon_hooks"] = mod
    try:
        from trn_agent_boot.trn_boot import _ntff_profile_via_ctypes
        h = _ntff_profile_via_ctypes("/opt/axon/libaxon_pjrt.so")
        if h is not None:
            mod.set_axon_ntff_profile_hook(h)
    except Exception:
        pass


def _get_compiled():
    global _COMPILED
    if _COMPILED is None:
        _COMPILED = _build_program()
    return _COMPILED


def kernel(x, T, _trace=False):
    if _trace:
        _ensure_ntff_hook()
    nc = _get_compiled()
    in_maps, MTb = _host_inputs(np.asarray(x, dtype=np.float32),
                                np.asarray(T, dtype=np.float32))
    res = bass_utils.run_bass_kernel_spmd(nc, in_maps,
                                          core_ids=list(range(NCORES)),
                                          trace=_trace)
    out = _assemble(res.results, MTb)
    if _trace:
        return out, res
    return out
